# revision 8
# baseline (speedup 1.0000x reference)
"""Trainium2 Bass kernel for nn_Decoder (latent MLP -> GRU scan -> per-step MLP).

Strategy: pure data-parallel over batch (4096 -> 8 x 512), weights replicated.
On-chip layout is fully transposed (feature dim on partitions, batch on free
dim). All recurrent weights are fp8 e4m3, pre-scaled by 8 on the host (the
activation un-scales with scale=1/8); h lives in [128, 2, 512] fp8
double-tiles. Matmuls whose consumers are DVE (n-gate hn) or cheap (pred
mm1/mm2) use DoubleRow perf mode (0.5 cycles/row, 2x PE throughput; outputs
are 64-partition PSUM memlocs, which DVE absorbs as half-ops but would halve
ACT throughput — so r/z gates stay non-DR fp8 with full [128,512] PSUM tiles
and single ACT sigmoids).

Self-contained: hardcodes shapes from the problem spec.
"""
import sys
sys.path.insert(0, "/opt/trn_rl_repo")
from contextlib import ExitStack

import numpy as np
import ml_dtypes

import concourse.bacc as bacc
import concourse.mybir as mybir
from concourse import tile
from concourse import bass_utils

BF16 = ml_dtypes.bfloat16
FP8 = ml_dtypes.float8_e4m3
BF = mybir.dt.bfloat16
F8 = mybir.dt.float8e4
F32 = mybir.dt.float32
AF = mybir.ActivationFunctionType
ALU = mybir.AluOpType
DR = mybir.MatmulPerfMode.DoubleRow

N_CORES = 8
B, LAT, H, A, L = 4096, 256, 512, 64, 128
BOS = 0
T = L - 1          # recurrence steps
BL = B // N_CORES  # per-core batch
KH = H // 128
G = H // 256       # DoubleRow K-groups
WS = 8.0           # host weight pre-scale (fp8 range); ACT applies 1/WS
IS = 1.0 / WS


def _build(steps=T, n_cores=N_CORES, reps=1, timing_iters=None, unroll=8,
           variant="full"):
    """Always declares the full-size DRAM interface (xT[T], y[:, L]); `steps`
    bounds the recurrence so short builds are wall-clock comparable.

    timing_iters: if set, wraps `unroll` statically-addressed step bodies in a
    hardware For_i loop executed timing_iters//unroll times (numerics garbage,
    per-step work identical) — used only to measure per-step device time."""
    nc = bacc.Bacc("TRN2", target_bir_lowering=False, debug=False,
                   num_devices=n_cores)

    d = {}
    def din(name, shape, dt=BF):
        d[name] = nc.dram_tensor(name, list(shape), dt, kind="ExternalInput").ap()

    din("latentT", [LAT, BL])
    din("xT", [T, A, BL])
    din("Whh8", [H, 3 * H], F8)
    din("WihT", [A, 3 * H])
    din("Wm18", [H, H], F8)
    din("Wm28", [H, A], F8)
    din("Wm3T", [A, A])
    din("Wd1T", [LAT, H])
    din("Wd2T", [H, H])
    din("Wd3T", [H, H])
    din("b_rz", [2 * H], F32)
    din("b_inn8", [H], F32)
    din("b_hnn8", [H], F32)
    din("bm1", [H], F32)
    din("bm2", [A], F32)
    din("bm3b", [128, A], F32)
    din("bd1", [H], F32)
    din("bd2", [H], F32)
    din("bd3", [H], F32)
    y = nc.dram_tensor("y", [BL, L, A], F32, kind="ExternalOutput").ap()

    with tile.TileContext(nc) as tc, ExitStack() as ctx:
        cst = ctx.enter_context(tc.tile_pool(name="const", bufs=1))
        wrk = ctx.enter_context(tc.tile_pool(name="work", bufs=2))
        hpool = ctx.enter_context(tc.tile_pool(name="hp", bufs=4))
        ps = ctx.enter_context(tc.tile_pool(name="ps", bufs=4, space="PSUM"))
        ph = ctx.enter_context(tc.tile_pool(name="ph", bufs=2, space="PSUM"))
        psy = ctx.enter_context(tc.tile_pool(name="psy", bufs=1, space="PSUM"))

        def const_tile(shape, dt, tag, src):
            t = cst.tile(list(shape), dt, tag=tag, name=tag)
            nc.sync.dma_start(t[:], src)
            return t

        def dr_weight(name, c0, cols, tag):
            """[128, 2, cols] fp8 tiles per K-group g: [:, i, :] holds source
            rows [256g+128i, 256g+128(i+1)), cols [c0, c0+cols)."""
            tiles = []
            for g in range(G):
                t = cst.tile([128, 2, cols], F8, tag=f"{tag}{g}",
                             name=f"{tag}{g}")
                for i in range(2):
                    nc.sync.dma_start(
                        t[:, i, :],
                        d[name][256 * g + 128 * i:256 * g + 128 * (i + 1),
                                c0:c0 + cols])
                tiles.append(t)
            return tiles

        whh8n = dr_weight("Whh8", 2 * H, H, "whhn")     # n-gate cols, DR
        wm18 = dr_weight("Wm18", 0, H, "wm1")
        wm28 = dr_weight("Wm28", 0, A, "wm2")
        # r/z gate cols, non-DR layout: per feature tile j, [128, 2H] fp8
        whrz = [const_tile([128, 2 * H], F8, f"whrz{j}",
                           d["Whh8"][j * 128:(j + 1) * 128, 0:2 * H])
                for j in range(KH)]
        wih = const_tile([A, 3 * H], BF, "wih", d["WihT"][:])
        wm3 = const_tile([A, A], BF, "wm3", d["Wm3T"][:])
        wd1 = [const_tile([128, H], BF, f"wd1{k}",
                          d["Wd1T"][k * 128:(k + 1) * 128, :]) for k in range(2)]
        wd2 = [const_tile([128, H], BF, f"wd2{k}",
                          d["Wd2T"][k * 128:(k + 1) * 128, :]) for k in range(KH)]
        wd3 = [const_tile([128, H], BF, f"wd3{k}",
                          d["Wd3T"][k * 128:(k + 1) * 128, :]) for k in range(KH)]

        def bias_tiles(name, n, tag):
            return [const_tile([128, 1], F32, f"{tag}{j}",
                               d[name][j * 128:(j + 1) * 128, None])
                    for j in range(n)]

        brz = bias_tiles("b_rz", 8, "brz")
        binn = bias_tiles("b_inn8", KH, "binn")
        bhnn = bias_tiles("b_hnn8", KH, "bhnn")
        bm1 = bias_tiles("bm1", KH, "bm1")
        bm2 = const_tile([A, 1], F32, "bm2", d["bm2"][:, None])
        bm3b = const_tile([128, A], F32, "bm3b", d["bm3b"][:])
        bd1 = bias_tiles("bd1", KH, "bd1")
        bd2 = bias_tiles("bd2", KH, "bd2")
        bd3 = bias_tiles("bd3", KH, "bd3")

        lat = [const_tile([128, BL], BF, f"lat{k}",
                          d["latentT"][k * 128:(k + 1) * 128, :]) for k in range(2)]

        def mlp_layer(w_tiles, rhs_tiles, bias, act, out_tag):
            outs = []
            for m in range(KH):
                acc = ps.tile([128, BL], F32, tag="ps", name="ps")
                nk = len(rhs_tiles)
                for k in range(nk):
                    nc.tensor.matmul(
                        acc[:], w_tiles[k][:, m * 128:(m + 1) * 128],
                        rhs_tiles[k][:], start=(k == 0), stop=(k == nk - 1))
                o = hpool.tile([128, BL], BF, tag=f"{out_tag}{m}",
                               name=f"{out_tag}{m}")
                nc.scalar.activation(o[:], acc[:], act, bias=bias[m][:])
                outs.append(o)
            return outs

        def new_h():
            return [hpool.tile([128, 2, BL], F8, tag=f"hg{g}", name=f"hg{g}")
                    for g in range(G)]

        h1 = mlp_layer(wd1, lat, bd1, AF.Tanh, "h1")
        h2 = mlp_layer(wd2, h1, bd2, AF.Tanh, "h2")
        hb0 = new_h()
        for m in range(KH):
            acc = ps.tile([128, BL], F32, tag="ps", name="ps")
            for k in range(KH):
                nc.tensor.matmul(
                    acc[:], wd3[k][:, m * 128:(m + 1) * 128],
                    h2[k][:], start=(k == 0), stop=(k == KH - 1))
            nc.scalar.activation(hb0[m // 2][:, m % 2, :], acc[:],
                                 AF.Identity, bias=bd3[m][:])

        # hist[t] = h double-tiles of step t (init state = hist[-1]); pred for
        # step t is emitted 2 steps later so its matmuls fill the PE stall
        # while the h(t-1) elementwise tail completes.
        state = {"hist": {-1: hb0}, "ystage": None}
        # variant flags (dev-only timing decomposition; graded path = "full")
        want_gates = variant in ("full", "nopred")
        want_pred = variant in ("full", "mmpred")
        want_mm = variant != "eltonly"

        def dr_half(acc, w_tiles, col0, hg):
            """One [64, BL] DR output memloc: columns in 256-chunks, K-groups
            accumulated. acc partition base must be 0 (ISA)."""
            for nq in range(2):
                for g in range(G):
                    nc.tensor.matmul(
                        acc[:, 256 * nq:256 * (nq + 1)],
                        w_tiles[g][:, :, col0:col0 + 64],
                        hg[g][:, :, 256 * nq:256 * (nq + 1)],
                        start=(g == 0), stop=(g == G - 1), perf_mode=DR)

        def nd_mm(acc, w_nd, col0, hg, xgate, xt):
            """Non-DR fp8 [128, BL] gate pre-activation: gi (bf16) + gh."""
            nc.tensor.matmul(acc[:], wih[:, xgate * 128:(xgate + 1) * 128],
                             xt[:], start=True, stop=False)
            for j in range(KH):
                nc.tensor.matmul(
                    acc[:], w_nd[j][:, col0:col0 + 128],
                    hg[j // 2][:, j % 2, :], start=False, stop=(j == KH - 1))

        def emit_gates(t):
            hg = state["hist"][t - 1]
            xt = wrk.tile([A, BL], BF, tag="xt", name="xt")
            nc.sync.dma_start(xt[:], d["xT"][t])

            if not want_mm:
                state["hist"][t] = hg
                return

            # r gates: non-DR fp8, full [128,512] psum, single sigmoid each
            r = []
            for m in range(KH):
                acc = ps.tile([128, BL], F32, tag="ps", name="ps")
                nd_mm(acc, whrz, 128 * m, hg, m, xt)
                if want_gates:
                    g = wrk.tile([128, BL], BF, tag=f"rz{m}", name=f"rz{m}")
                    nc.scalar.activation(g[:], acc[:], AF.Sigmoid,
                                         bias=brz[m][:], scale=IS)
                    r.append(g)

            # n-gate per j: hn via DR (2 half-memlocs), inn via gi; z-gate
            # matmuls interleave so PE never waits on the rhn consumption
            n_list, d_list, z_list = [], [], []
            for j in range(KH):
                hn0 = ph.tile([64, BL], F32, tag="ph", name="ph")
                hn1 = ph.tile([64, BL], F32, tag="ph", name="ph")
                dr_half(hn0, whh8n, 128 * j, hg)
                dr_half(hn1, whh8n, 128 * j + 64, hg)
                inn = ps.tile([128, BL], F32, tag="ps", name="ps")
                gi_m = 8 + j
                nc.tensor.matmul(acc_z := ps.tile([128, BL], F32, tag="ps",
                                                  name="ps"),
                                 wih[:, (4 + j) * 128:(5 + j) * 128],
                                 xt[:], start=True, stop=False)
                nc.tensor.matmul(inn[:], wih[:, gi_m * 128:(gi_m + 1) * 128],
                                 xt[:], start=True, stop=True)
                for k in range(KH):
                    nc.tensor.matmul(
                        acc_z[:], whrz[k][:, (4 + j) * 128:(5 + j) * 128],
                        hg[k // 2][:, k % 2, :], start=False,
                        stop=(k == KH - 1))
                if not want_gates:
                    continue
                z = wrk.tile([128, BL], BF, tag=f"rz{4+j}", name=f"rz{4+j}")
                nc.scalar.activation(z[:], acc_z[:], AF.Sigmoid,
                                     bias=brz[4 + j][:], scale=IS)
                z_list.append(z)
                rhn = wrk.tile([128, BL], F32, tag="rhn", name="rhn")
                for q, hnq in enumerate((hn0, hn1)):
                    nc.vector.scalar_tensor_tensor(
                        rhn[64 * q:64 * (q + 1), :], hnq[:],
                        bhnn[j][64 * q:64 * (q + 1), :],
                        r[j][64 * q:64 * (q + 1), :],
                        op0=ALU.add, op1=ALU.mult)
                s = wrk.tile([128, BL], F32, tag="s", name="s")
                nc.vector.scalar_tensor_tensor(
                    s[:], inn[:], binn[j][:], rhn[:],
                    op0=ALU.add, op1=ALU.add)
                n_t = wrk.tile([128, BL], BF, tag="nt", name="nt")
                nc.scalar.activation(n_t[:], s[:], AF.Tanh, scale=IS)
                d_t = wrk.tile([128, BL], BF, tag="dt", name="dt")
                nc.gpsimd.tensor_sub(d_t[:], hg[j // 2][:, j % 2, :], n_t[:])
                n_list.append(n_t)
                d_list.append(d_t)

            hg_new = new_h() if want_gates else hg
            if want_gates:
                for j in range(KH):
                    zd = wrk.tile([128, BL], BF, tag="zd", name="zd")
                    nc.vector.tensor_mul(zd[:], z_list[j][:], d_list[j][:])
                    nc.vector.tensor_add(hg_new[j // 2][:, j % 2, :],
                                         n_list[j][:], zd[:])
            state["hist"][t] = hg_new

        def emit_pred(tp, last, force=False):
            if not want_pred or (tp < 0 and not force):
                return
            hg = (state["hist"][tp] if tp in state["hist"]
                  else state["hist"][-1])
            ystage = state["ystage"]
            pg = [wrk.tile([128, 2, BL], F8, tag=f"pg{g}", name=f"pg{g}")
                  for g in range(G)]
            for m in range(KH):
                for mh in range(2):
                    acc = ph.tile([64, BL], F32, tag="ph", name="ph")
                    if want_mm:
                        dr_half(acc, wm18, 128 * m + 64 * mh, hg)
                    nc.scalar.activation(
                        pg[m // 2][64 * mh:64 * (mh + 1), m % 2, :], acc[:],
                        AF.Tanh, bias=bm1[m][64 * mh:64 * (mh + 1), :],
                        scale=IS)
            acc2 = ph.tile([A, BL], F32, tag="ph", name="ph")
            if want_mm:
                dr_half(acc2, wm28, 0, pg)
            p2 = wrk.tile([A, BL], BF, tag="p2", name="p2")
            nc.scalar.activation(p2[:], acc2[:], AF.Tanh, bias=bm2[:],
                                 scale=IS)

            tps = tp if tp >= 0 else tp + 8  # timing-build pseudo-slot
            o = (tps + 1) % 8
            g = (tps + 1) // 8
            if ystage is None or o == 0 or (g == 0 and o == 1):
                ystage = [wrk.tile([128, 8 * A], F32, tag=f"yst{bt}",
                                   name=f"yst{bt}") for bt in range(4)]
            for bt in range(4):
                yp = psy.tile([128, A], F32, tag="psy", name="psy")
                nc.tensor.matmul(yp[:], p2[:, bt * 128:(bt + 1) * 128],
                                 wm3[:], start=True, stop=True)
                nc.vector.tensor_add(
                    ystage[bt][:, o * A:(o + 1) * A], yp[:], bm3b[:])
            if o == 7 or last:
                lo = 1 if g == 0 else 0
                hi = o + 1
                for bt in range(4):
                    nc.sync.dma_start(
                        y[bt * 128:(bt + 1) * 128, g * 8 + lo:g * 8 + hi, :],
                        ystage[bt][:, lo * A:hi * A])
            state["ystage"] = ystage
            # h(tp) no longer needed once its pred is done
            state["hist"].pop(tp - 1, None)

        PRED_LAG = 2
        if timing_iters is None:
            for _rep in range(reps):
                for t in range(steps):
                    emit_pred(t - PRED_LAG, last=False)
                    emit_gates(t)
                for tp in range(max(steps - PRED_LAG, 0), steps):
                    emit_pred(tp, last=(tp == steps - 1))
        else:
            # timing loop: same per-step work (preds for t<LAG read init h —
            # numerics are garbage in timing builds anyway)
            with tc.For_i(0, timing_iters // unroll, 1):
                for t in range(unroll):
                    emit_pred(t - PRED_LAG, last=False, force=True)
                    emit_gates(t)

    nc.compile()
    return nc


def _make_bos():
    bos = np.full((B, A), -16.0, np.float32)
    bos[:, BOS] = 16.0
    return bos


def _make_in_maps(inputs, n_cores=N_CORES, T=T):
    bl = B // n_cores
    f32 = np.float32
    def t8(w):  # transpose, scale by WS, cast fp8
        return np.ascontiguousarray(
            np.asarray(w, f32).T * WS).astype(FP8)
    shared = {
        "Whh8": t8(inputs["W_hh"]),
        "WihT": np.ascontiguousarray(
            np.asarray(inputs["W_ih"], f32).T * WS).astype(BF16),
        "Wm18": t8(inputs["Wm1"]),
        "Wm28": t8(inputs["Wm2"]),
        "Wm3T": np.ascontiguousarray(np.asarray(inputs["Wm3"], f32).T).astype(BF16),
        "Wd1T": np.ascontiguousarray(np.asarray(inputs["Wd1"], f32).T).astype(BF16),
        "Wd2T": np.ascontiguousarray(np.asarray(inputs["Wd2"], f32).T).astype(BF16),
        "Wd3T": np.ascontiguousarray(np.asarray(inputs["Wd3"], f32).T).astype(BF16),
        "b_rz": (np.asarray(inputs["b_ih"], f32)
                 + np.asarray(inputs["b_hh"], f32))[:2 * H].astype(f32),
        "b_inn8": (np.asarray(inputs["b_ih"], f32)[2 * H:] * WS).astype(f32),
        "b_hnn8": (np.asarray(inputs["b_hh"], f32)[2 * H:] * WS).astype(f32),
        "bm1": np.asarray(inputs["bm1"], f32),
        "bm2": np.asarray(inputs["bm2"], f32),
        "bm3b": np.ascontiguousarray(
            np.broadcast_to(np.asarray(inputs["bm3"], f32), (128, A))),
        "bd1": np.asarray(inputs["bd1"], f32),
        "bd2": np.asarray(inputs["bd2"], f32),
        "bd3": np.asarray(inputs["bd3"], f32),
    }
    bos = _make_bos()
    latent = np.asarray(inputs["latent"], f32)
    target = np.asarray(inputs["target"], f32)
    in_maps = []
    for c in range(n_cores):
        sl = slice(c * bl, (c + 1) * bl)
        xT = np.empty((T, A, bl), BF16)
        xT[0] = bos[sl].T
        if T > 1:
            xT[1:] = target[sl, 1:T].transpose(1, 2, 0).astype(BF16)
        m = dict(shared)
        m["latentT"] = np.ascontiguousarray(latent[sl].T).astype(BF16)
        m["xT"] = xT
        in_maps.append(m)
    return in_maps


_NC_CACHE = {}


def _get_nc(steps=T, reps=1):
    key = (steps, reps)
    if key not in _NC_CACHE:
        _NC_CACHE[key] = _build(steps=steps, reps=reps)
    return _NC_CACHE[key]


def kernel(**inputs):
    nc = _get_nc()
    in_maps = _make_in_maps(inputs)
    res = bass_utils.run_bass_kernel_spmd(nc, in_maps,
                                          core_ids=list(range(N_CORES)))
    bl = B // N_CORES
    y = np.empty((B, L, A), np.float32)
    for c in range(N_CORES):
        y[c * bl:(c + 1) * bl] = res.results[c]["y"]
    y[:, 0, :] = _make_bos()
    return y
